# revision 12
# baseline (speedup 1.0000x reference)
"""BigBird block-sparse attention on 8 Trainium2 NeuronCores.

Sharding: core c handles batch b = c // 4 and 3 heads starting at 3 * (c % 4).
Each core computes its partial output projection OUT_c = sum_h ctx_h @ WoT_h
(fp32, [4096, 768]); the host sums the 4 partials per batch and adds the
bias terms (Wo_b plus the Wv bias pushed through the output projection).

Device-side layout (per core, all compute bf16, accum fp32):
  X [4096,768] -> XT [768,4096] via PE transposes -> projections produce
  qT/kT ([hd, s], heads 0,1 stacked on 128 partitions; head 2 on 64),
  V in natural [s, hd] layout (plus a 64-row-shifted copy V2 so that any
  key block is available at either partition parity).
  Per query-block pair (2 blocks stacked on 128 partitions): 8 score
  matmuls per block against the statically-known gathered key blocks
  (rand_idx is baked into the program at build time), exp via ACT with
  accum_out giving the softmax denominator, per-partition normalize on
  DVE, PE transposes to get probsT, then 8 accumulating matmuls
  (lhsT = V block) produce ctxT directly. Global query blocks 0/63 are
  recomputed densely against all 4096 keys and overwrite the sparse
  result. Output projection contracts ctxT against WoT.
"""

import numpy as np
import ml_dtypes

B, S, HID = 2, 4096, 768
NH, HD = 12, 64
BS = 64
NB = S // BS            # 64 blocks
NR = 3
K = 8                   # gathered key blocks per query block
NCORES = 8
HPC = 3                 # heads per core
NCHUNK = S // 128       # 32 s-chunks of 128
NK6 = HID // 128        # 6 hid-chunks

BF16 = ml_dtypes.bfloat16

_cache = {}
_last_in_maps = None


def _build(idx, upto="G", dsub=0):
    """Build the SPMD Bass program. idx: tuple of NB tuples of K key-block ids.

    upto: last stage to include ("B", "C", "D", "G") — for bisection.
    dsub: if >0, include only the first `dsub` sub-steps of stage D
          (1=scores+exp, 2=+normalize, 3=+transposes, 4=full) and write
          a dummy output.
    """
    import concourse.mybir as mybir
    from concourse import bacc
    from concourse.tile import TileContext

    dt = mybir.dt
    order = {"B": 0, "C": 1, "D": 2, "G": 3}
    lvl = order[upto]
    nc = bacc.Bacc()

    # ---- DRAM tensors ----
    X_d = nc.dram_tensor("X", [S, HID], dt.bfloat16, kind="ExternalInput")
    WQT_d = nc.dram_tensor("WQT", [HID, 192], dt.bfloat16, kind="ExternalInput")
    WKT_d = nc.dram_tensor("WKT", [HID, 192], dt.bfloat16, kind="ExternalInput")
    WVT_d = nc.dram_tensor("WVT", [HID, 192], dt.bfloat16, kind="ExternalInput")
    WOT_d = nc.dram_tensor("WOT", [192, HID], dt.bfloat16, kind="ExternalInput")
    BQ_d = nc.dram_tensor("BQ", [192, 1], dt.float32, kind="ExternalInput")
    BK_d = nc.dram_tensor("BK", [192, 1], dt.float32, kind="ExternalInput")
    ID_d = nc.dram_tensor("IDN", [128, 128], dt.bfloat16, kind="ExternalInput")
    OUT_d = nc.dram_tensor("OUT", [S, HID], dt.float32, kind="ExternalOutput")

    with TileContext(nc) as tc:
        with tc.tile_pool(name="persist", bufs=1) as pers:
            # ---- persistent SBUF tiles ----
            XT = pers.tile([128, NK6 * S], dt.bfloat16)
            qT01 = pers.tile([128, S], dt.bfloat16)
            qT2 = pers.tile([64, S], dt.bfloat16)
            kT01 = pers.tile([128, S], dt.bfloat16)
            kT2 = pers.tile([64, S], dt.bfloat16)
            Vs = pers.tile([128, NCHUNK * 192], dt.bfloat16)
            V2 = pers.tile([128, (NCHUNK + 1) * 192], dt.bfloat16)
            ctxT01 = pers.tile([128, S], dt.bfloat16)
            ctxT2 = pers.tile([64, S], dt.bfloat16)
            WQT = pers.tile([128, NK6 * 192], dt.bfloat16)
            WKT = pers.tile([128, NK6 * 192], dt.bfloat16)
            WVT = pers.tile([128, NK6 * 192], dt.bfloat16)
            WOT01 = pers.tile([128, HID], dt.bfloat16)
            WOT2 = pers.tile([64, HID], dt.bfloat16)
            BQ01 = pers.tile([128, 1], dt.float32)
            BQ2 = pers.tile([64, 1], dt.float32)
            BK01 = pers.tile([128, 1], dt.float32)
            BK2 = pers.tile([64, 1], dt.float32)
            IDN = pers.tile([128, 128], dt.bfloat16)

            # ---- load weights/constants ----
            for wt_sb, wt_d in ((WQT, WQT_d), (WKT, WKT_d), (WVT, WVT_d)):
                nc.sync.dma_start(
                    wt_sb[:].rearrange("p (c n) -> p c n", c=NK6),
                    wt_d.ap().rearrange("(c p) n -> p c n", p=128),
                )
            nc.sync.dma_start(WOT01[:], WOT_d[0:128, :])
            nc.sync.dma_start(WOT2[:], WOT_d[128:192, :])
            nc.sync.dma_start(BQ01[:], BQ_d[0:128, :])
            nc.sync.dma_start(BQ2[:], BQ_d[128:192, :])
            nc.sync.dma_start(BK01[:], BK_d[0:128, :])
            nc.sync.dma_start(BK2[:], BK_d[128:192, :])
            nc.sync.dma_start(IDN[:], ID_d[:, :])

            # ---- stages A+B: load X, build XT via PE transposes ----
            with (
                tc.tile_pool(name="xin", bufs=1) as xin_pool,
                tc.tile_pool(name="psB", bufs=2, space="PSUM") as psB,
            ):
                X_sb = xin_pool.tile([128, NCHUNK * HID], dt.bfloat16)
                for t in range(NCHUNK):
                    nc.sync.dma_start(
                        X_sb[:, t * HID : (t + 1) * HID],
                        X_d[t * 128 : (t + 1) * 128, :],
                    )
                for c6 in range(NK6):
                    for tg in range(NCHUNK // 4):
                        pt = psB.tile([128, 512], dt.bfloat16, tag="pT")
                        for i in range(4):
                            t = tg * 4 + i
                            nc.tensor.transpose(
                                pt[:, i * 128 : (i + 1) * 128],
                                X_sb[:, t * HID + c6 * 128 : t * HID + (c6 + 1) * 128],
                                IDN[:],
                            )
                        nc.vector.tensor_copy(
                            XT[:, c6 * S + tg * 512 : c6 * S + (tg + 1) * 512], pt[:]
                        )

            # ---- stage C: projections ----
            def proj_qk(WT, dst, bias, m0, m1):
                M = m1 - m0
                with tc.tile_pool(name="psC", bufs=8, space="PSUM") as psC:
                    pts = [
                        psC.tile([M, 512], dt.float32, tag="pp", name=f"pp{nb}")
                        for nb in range(8)
                    ]
                    for c6 in range(NK6):
                        lhs = WT[:, c6 * 192 + m0 : c6 * 192 + m1]
                        for nb in range(8):
                            nc.tensor.matmul(
                                pts[nb][:],
                                lhs,
                                XT[:, c6 * S + nb * 512 : c6 * S + (nb + 1) * 512],
                                start=(c6 == 0),
                                stop=(c6 == NK6 - 1),
                            )
                    for nb in range(8):
                        nc.scalar.activation(
                            dst[:, nb * 512 : (nb + 1) * 512],
                            pts[nb][:],
                            mybir.ActivationFunctionType.Identity,
                            bias=bias[:],
                            scale=1.0,
                        )

            if lvl >= 1:
                proj_qk(WQT, qT01, BQ01, 0, 128)
                proj_qk(WQT, qT2, BQ2, 128, 192)
                proj_qk(WKT, kT01, BK01, 0, 128)
                proj_qk(WKT, kT2, BK2, 128, 192)

                # V natural
                with tc.tile_pool(name="psV", bufs=4, space="PSUM") as psV:
                    for t in range(NCHUNK):
                        pv = psV.tile([128, 192], dt.float32, tag="pv")
                        for c6 in range(NK6):
                            nc.tensor.matmul(
                                pv[:],
                                XT[:, c6 * S + t * 128 : c6 * S + (t + 1) * 128],
                                WVT[:, c6 * 192 : (c6 + 1) * 192],
                                start=(c6 == 0),
                                stop=(c6 == NK6 - 1),
                            )
                        nc.vector.tensor_copy(Vs[:, t * 192 : (t + 1) * 192], pv[:])
                # V2: odd key blocks copied to partitions 0-63
                # (SB->SB DMA crosses partitions)
                for t in range(NCHUNK):
                    nc.sync.dma_start(
                        V2[0:64, (t + 1) * 192 : (t + 2) * 192],
                        Vs[64:128, t * 192 : (t + 1) * 192],
                    )

            # helpers
            def qT(h):
                return qT01 if h < 2 else qT2

            def kT(h):
                return kT01 if h < 2 else kT2

            def qbase(h):
                return 64 * h if h < 2 else 0

            def v_ap0(j, h):
                """V rows of key block j at partitions 0-63, head h cols."""
                if j % 2 == 0:
                    t = j // 2
                    return Vs[0:64, t * 192 + h * 64 : t * 192 + h * 64 + 64]
                u = (j + 1) // 2
                return V2[0:64, u * 192 + h * 64 : u * 192 + h * 64 + 64]

            ctxT = {0: ctxT01, 1: ctxT01, 2: ctxT2}

            def ctx_base(h):
                return 64 * h if h < 2 else 0

            # ---- stages D+E: attention ----
            if lvl >= 2:
                with (
                    tc.tile_pool(name="psS", bufs=2, space="PSUM") as psS,
                    tc.tile_pool(name="psT", bufs=2, space="PSUM") as psT,
                    tc.tile_pool(name="psX", bufs=2, space="PSUM") as psX,
                    tc.tile_pool(name="sprob", bufs=3) as sprob,
                    tc.tile_pool(name="small", bufs=4) as small,
                ):
                    # stage D: sparse blocks
                    for t in range(NB // 2):
                        for h in range(HPC):
                            qt, kt = qT(h), kT(h)
                            qb = qbase(h)
                            nA, nB_ = 2 * t, 2 * t + 1
                            ps = psS.tile([128, 512], dt.float32, tag="sc")
                            for half, n in ((0, nA), (1, nB_)):
                                lhs = qt[qb : qb + 64, n * 64 : (n + 1) * 64]
                                for slot in range(K):
                                    j = idx[n][slot]
                                    nc.tensor.matmul(
                                        ps[half * 64 : half * 64 + 64,
                                           slot * 64 : (slot + 1) * 64],
                                        lhs,
                                        kt[qb : qb + 64, j * 64 : (j + 1) * 64],
                                    )
                            probs = sprob.tile([128, 512], dt.bfloat16, tag="pr")
                            lcol = small.tile([128, 1], dt.float32, tag="l")
                            nc.scalar.activation(
                                probs[:], ps[:],
                                mybir.ActivationFunctionType.Exp,
                                accum_out=lcol[:],
                            )
                            if dsub == 1:
                                continue
                            rcol = small.tile([128, 1], dt.float32, tag="r")
                            nc.vector.reciprocal(rcol[:], lcol[:])
                            nc.vector.tensor_scalar_mul(probs[:], probs[:], rcol[:])
                            if dsub == 2:
                                continue
                            pT = psT.tile([64, K * 128], dt.bfloat16, tag="pt")
                            for slot in range(K):
                                nc.tensor.transpose(
                                    pT[:, slot * 128 : (slot + 1) * 128],
                                    probs[:, slot * 64 : (slot + 1) * 64],
                                    IDN[:],
                                )
                            probsT = sprob.tile([64, K * 128], dt.bfloat16, tag="pT2")
                            nc.vector.tensor_copy(probsT[:], pT[:])
                            if dsub == 3:
                                continue
                            pcs = [
                                psX.tile([64, 64], dt.float32, tag=f"cx{i}",
                                         name=f"pc{i}")
                                for i in range(2)
                            ]
                            for half, n in ((0, nA), (1, nB_)):
                                for slot in range(K):
                                    j = idx[n][slot]
                                    rhs = probsT[:, slot * 128 + half * 64
                                                 : slot * 128 + half * 64 + 64]
                                    nc.tensor.matmul(
                                        pcs[half][:],
                                        v_ap0(j, h),
                                        rhs,
                                        start=(slot == 0),
                                        stop=(slot == K - 1),
                                    )
                            cb = ctx_base(h)
                            dst = ctxT[h]
                            nc.vector.tensor_copy(
                                dst[cb : cb + 64, nA * 64 : (nA + 1) * 64],
                                pcs[0][:],
                            )
                            nc.vector.tensor_copy(
                                dst[cb : cb + 64, nB_ * 64 : (nB_ + 1) * 64],
                                pcs[1][:],
                            )

                    # stage E: global rows (blocks 0 and NB-1), dense
                    if lvl >= 3 and dsub == 0:
                        with tc.tile_pool(name="gprob", bufs=2) as gprob:
                            for h in range(HPC):
                                qt, kt = qT(h), kT(h)
                                qb = qbase(h)
                                for gn in (0, NB - 1):
                                    lhs = qt[qb : qb + 64, gn * 64 : (gn + 1) * 64]
                                    gp = gprob.tile([64, S], dt.bfloat16, tag="gp")
                                    l8 = small.tile([64, 8], dt.float32, tag="l8")
                                    for kb in range(8):
                                        ps = psS.tile([64, 512], dt.float32, tag="sc")
                                        nc.tensor.matmul(
                                            ps[:], lhs,
                                            kt[qb : qb + 64, kb * 512 : (kb + 1) * 512],
                                        )
                                        nc.scalar.activation(
                                            gp[:, kb * 512 : (kb + 1) * 512], ps[:],
                                            mybir.ActivationFunctionType.Exp,
                                            accum_out=l8[:, kb : kb + 1],
                                        )
                                    lg = small.tile([64, 1], dt.float32, tag="l")
                                    nc.vector.reduce_sum(
                                        lg[:], l8[:], axis=mybir.AxisListType.X
                                    )
                                    rg = small.tile([64, 1], dt.float32, tag="r")
                                    nc.vector.reciprocal(rg[:], lg[:])
                                    nc.vector.tensor_scalar_mul(gp[:], gp[:], rg[:])
                                    pcg = psX.tile([64, 64], dt.float32, tag="cx0")
                                    for cg in range(8):
                                        pT = psT.tile([128, 256], dt.bfloat16, tag="pt")
                                        for i in range(4):
                                            c = cg * 4 + i
                                            nc.tensor.transpose(
                                                pT[:, i * 64 : (i + 1) * 64],
                                                gp[:, c * 128 : (c + 1) * 128],
                                                IDN[0:64, 0:64],
                                            )
                                        probsTg = sprob.tile(
                                            [128, 256], dt.bfloat16, tag="pT2"
                                        )
                                        nc.vector.tensor_copy(probsTg[:], pT[:])
                                        for i in range(4):
                                            c = cg * 4 + i
                                            nc.tensor.matmul(
                                                pcg[:],
                                                Vs[:, c * 192 + h * 64
                                                   : c * 192 + h * 64 + 64],
                                                probsTg[:, i * 64 : (i + 1) * 64],
                                                start=(c == 0),
                                                stop=(c == NCHUNK - 1),
                                            )
                                    cb = ctx_base(h)
                                    nc.vector.tensor_copy(
                                        ctxT[h][cb : cb + 64, gn * 64 : (gn + 1) * 64],
                                        pcg[:],
                                    )

            # ---- stage G: output projection (or dummy output for bisection) ----
            with (
                tc.tile_pool(name="psO", bufs=4, space="PSUM") as psO,
                tc.tile_pool(name="osb", bufs=3) as osb,
            ):
                if lvl >= 2 and dsub == 0:
                    for t in range(NCHUNK):
                        pos = [
                            psO.tile([128, 384], dt.float32, tag="po", name=f"po{i}")
                            for i in range(2)
                        ]
                        for nh in range(2):
                            nc.tensor.matmul(
                                pos[nh][:],
                                ctxT01[:, t * 128 : (t + 1) * 128],
                                WOT01[:, nh * 384 : (nh + 1) * 384],
                                start=True, stop=False,
                            )
                            nc.tensor.matmul(
                                pos[nh][:],
                                ctxT2[:, t * 128 : (t + 1) * 128],
                                WOT2[:, nh * 384 : (nh + 1) * 384],
                                start=False, stop=True,
                            )
                        ot = osb.tile([128, HID], dt.float32, tag="ot")
                        for nh in range(2):
                            nc.vector.tensor_copy(
                                ot[:, nh * 384 : (nh + 1) * 384], pos[nh][:]
                            )
                        nc.sync.dma_start(OUT_d[t * 128 : (t + 1) * 128, :], ot[:])
                else:
                    zt = osb.tile([128, HID], dt.float32, tag="ot")
                    nc.vector.memset(zt[:], 0.0)
                    for t in range(NCHUNK):
                        nc.sync.dma_start(OUT_d[t * 128 : (t + 1) * 128, :], zt[:])

    nc.finalize()
    return nc


def kernel(X, band_mask, from_mask, to_mask, blocked_encoder_mask, rand_idx,
           Wq_w, Wq_b, Wk_w, Wk_b, Wv_w, Wv_b, Wo_w, Wo_b):
    from concourse.bass_utils import run_bass_kernel_spmd

    X = np.asarray(X, dtype=np.float32)
    rand_idx = np.asarray(rand_idx)
    Wq_w = np.asarray(Wq_w, np.float32); Wq_b = np.asarray(Wq_b, np.float32)
    Wk_w = np.asarray(Wk_w, np.float32); Wk_b = np.asarray(Wk_b, np.float32)
    Wv_w = np.asarray(Wv_w, np.float32); Wv_b = np.asarray(Wv_b, np.float32)
    Wo_w = np.asarray(Wo_w, np.float32); Wo_b = np.asarray(Wo_b, np.float32)

    blk = np.arange(NB)
    window = (blk[:, None] + np.array([-1, 0, 1])[None, :]) % NB
    glob = np.broadcast_to(np.array([0, NB - 1]), (NB, 2))
    idx = np.concatenate([window, glob, rand_idx.astype(np.int64)], axis=1)
    import os
    upto = os.environ.get("KERNEL_UPTO", "G")
    dsub = int(os.environ.get("KERNEL_DSUB", "0"))
    key = (idx.tobytes(), upto, dsub)
    if key not in _cache:
        _cache[key] = _build(
            tuple(tuple(int(v) for v in row) for row in idx), upto=upto, dsub=dsub
        )
    nc = _cache[key]

    sc = 1.0 / np.sqrt(HD)
    identity = np.eye(128, dtype=np.float32).astype(BF16)
    in_maps = []
    for c in range(NCORES):
        b = c // 4
        h0 = HPC * (c % 4)
        hsl = slice(h0 * HD, (h0 + HPC) * HD)
        in_maps.append({
            "X": X[b].astype(BF16),
            "WQT": np.ascontiguousarray((Wq_w[hsl, :] * sc).T).astype(BF16),
            "WKT": np.ascontiguousarray(Wk_w[hsl, :].T).astype(BF16),
            "WVT": np.ascontiguousarray(Wv_w[hsl, :].T).astype(BF16),
            "WOT": np.ascontiguousarray(Wo_w[:, hsl].T).astype(BF16),
            "BQ": (Wq_b[hsl] * sc).astype(np.float32)[:, None],
            "BK": Wk_b[hsl].astype(np.float32)[:, None],
            "IDN": identity,
        })

    global _last_in_maps
    _last_in_maps = in_maps
    res = run_bass_kernel_spmd(nc, in_maps, core_ids=list(range(NCORES)))

    out = np.zeros((B, S, HID), dtype=np.float32)
    for c in range(NCORES):
        out[c // 4] += res.results[c]["OUT"]
    # bias terms handled on host: Wo bias, and Wv bias pushed through Wo
    # (sum_k probs = 1, so ctx picks up Wv_b exactly).
    out += (Wo_w @ Wv_b + Wo_b)[None, None, :]
    fm = np.asarray(from_mask, np.float32).reshape(B, S)
    if not np.all(fm == 1.0):
        raise NotImplementedError("kernel assumes all-ones from_mask")
    return out
